# revision 6
# baseline (speedup 1.0000x reference)
"""Causal self-attention (B=4, T=2048, C=1024, H=16) on 8 trn2 NeuronCores.

Sharding: token-parallel, zero collectives. Core c handles batch b = c//2,
token parity p = c%2 (tokens p::2 of that batch, 1024 queries). Every core
computes full K/V for its batch (2048 keys); causality means the strided
token split gives every core an identical loop structure (uniform SPMD
program), with the per-core causal boundary expressed purely as input data
(a [512, 256] mask tile that is the same for every q-tile).

Per-core pipeline (all layouts chosen so no transposes are ever needed):
  A) QKV projection (bf16):  Q^T/K^T feature-major via lhsT=W chunk,
     rhs=x^T chunk; V token-major via lhsT=x^T chunk, rhs=W_v chunk.
  B) Attention: S^T[k,q] = K^T.T @ Q^T (even/odd heads auto row-packed on
     the PE via partition offsets 0/64); P^T = exp(S^T * 0.125) on ScalarE
     straight out of PSUM into bf16 SBUF; causal mask = bf16 multiply on
     the 4 diagonal k-tiles only; O^T accumulated in PSUM via lhsT=V
     (col-tiled: even head cols 0-63, odd cols 64-127 of the PE array) and
     the softmax denominator l accumulated concurrently via a 1-column
     ones matmul; normalization r=1/l broadcast across partitions with a
     K=1 matmul (ones[1,64] outer r[1,q]).
  C) Output projection (float32r): out^T[c_out,q] = W_proj.T-free form
     lhsT=W_proj chunk, rhs=y^T chunk. Host reassembles out[b, p::2, :].
"""

import os
import sys

import numpy as np

for _p in ("/opt/trn_rl_repo",):
    if os.path.isdir(_p) and _p not in sys.path:
        sys.path.insert(0, _p)

import ml_dtypes

B, T, C, H = 4, 2048, 1024, 16
HD = C // H  # 64
P = 128
CI = C // P  # 8 contraction chunks
NCORE = 8
TQ = T // 2  # 1024 local queries per core
QB = 256  # q-block (matmul moving free dim)
NQB = TQ // QB  # 4
NKT = T // P  # 16 k-tiles
BF16 = ml_dtypes.bfloat16

_CACHE = {}
LAST_RESULTS = None


def _build():
    """Build + compile the (single, uniform) bass module once."""
    from contextlib import ExitStack

    import concourse.bass as bass  # noqa: F401
    import concourse.mybir as mybir
    from concourse import bacc, tile

    dt = mybir.dt
    f32, bf16, f32r = dt.float32, dt.bfloat16, dt.float32r
    EXP = mybir.ActivationFunctionType.Exp

    nc = bacc.Bacc(
        "TRN2",
        target_bir_lowering=False,
        debug=False,
        enable_asserts=False,
        num_devices=NCORE,
    )
    xt = nc.dram_tensor("xt", [C, T], bf16, kind="ExternalInput").ap()
    xtq = nc.dram_tensor("xtq", [C, TQ], bf16, kind="ExternalInput").ap()
    wa = nc.dram_tensor("wa", [C, 3 * C], bf16, kind="ExternalInput").ap()
    wp = nc.dram_tensor("wp", [C, C], f32r, kind="ExternalInput").ap()
    mk = nc.dram_tensor("mk", [4 * P, QB], bf16, kind="ExternalInput").ap()
    out_t = nc.dram_tensor("out_t", [C, TQ], f32, kind="ExternalOutput").ap()

    with tile.TileContext(nc) as tc, ExitStack() as ctx:
        # Long-lived tiles (QKV results + constants), freed before phase C.
        res = ctx.enter_context(tc.tile_pool(name="res", bufs=1))
        yt_pool = ctx.enter_context(tc.tile_pool(name="ytp", bufs=1))
        KT = res.tile([P, CI, T], bf16, name="KT")
        QT = res.tile([P, CI, TQ], bf16, name="QT")
        V = res.tile([P, NKT, H, HD], bf16, name="Vt")
        maskS = res.tile([P, 4, QB], bf16, name="maskS")
        ones_col = res.tile([P, 1], bf16, name="ones_col")
        yT = yt_pool.tile([P, CI, TQ], f32r, name="yT")

        nc.sync.dma_start(maskS, mk.rearrange("(o p) q -> p o q", p=P))
        nc.gpsimd.memset(ones_col, 1.0)

        # ---------------- Phase A: QKV projection (bf16) ----------------
        # A1: Q^T [feat, q] for this core's 1024 query tokens.
        with tc.tile_pool(name="xtqp", bufs=1) as xtqp, \
             tc.tile_pool(name="wqp", bufs=2) as wqp, \
             tc.tile_pool(name="psA", bufs=4, space="PSUM") as psA:
            xtq_sb = xtqp.tile([P, CI, TQ], bf16, name="xtq_sb")
            nc.sync.dma_start(xtq_sb, xtq.rearrange("(o p) t -> p o t", p=P))
            for m in range(CI):
                wq_sb = wqp.tile([P, CI, P], bf16, name="wq_sb")
                nc.sync.dma_start(
                    wq_sb, wa[:, m * P:(m + 1) * P].rearrange("(o p) f -> p o f", p=P)
                )
                for tb in range(TQ // 512):
                    ps = psA.tile([P, 512], f32, name="psA_t")
                    for ci in range(CI):
                        nc.tensor.matmul(
                            ps,
                            lhsT=wq_sb[:, ci],
                            rhs=xtq_sb[:, ci, tb * 512:(tb + 1) * 512],
                            start=(ci == 0),
                            stop=(ci == CI - 1),
                        )
                    nc.vector.tensor_copy(QT[:, m, tb * 512:(tb + 1) * 512], ps)

        # A2: K^T [feat, k] and V [k, feat] for all 2048 tokens of the batch.
        with tc.tile_pool(name="wkvp", bufs=1) as wkvp, \
             tc.tile_pool(name="xtp", bufs=2) as xtp, \
             tc.tile_pool(name="psA2", bufs=4, space="PSUM") as psA2:
            wkv_sb = wkvp.tile([P, CI, 2 * C], bf16, name="wkv_sb")
            nc.sync.dma_start(wkv_sb, wa[:, C:].rearrange("(o p) f -> p o f", p=P))
            for th in range(T // 512):  # token quarters
                xt_sb = xtp.tile([P, CI, 512], bf16, name="xt_sb")
                nc.sync.dma_start(
                    xt_sb, xt[:, th * 512:(th + 1) * 512].rearrange("(o p) t -> p o t", p=P)
                )
                for m in range(CI):  # K^T feature chunks
                    ps = psA2.tile([P, 512], f32, name="psA2_t")
                    for ci in range(CI):
                        nc.tensor.matmul(
                            ps,
                            lhsT=wkv_sb[:, ci, m * P:(m + 1) * P],
                            rhs=xt_sb[:, ci],
                            start=(ci == 0),
                            stop=(ci == CI - 1),
                        )
                    nc.vector.tensor_copy(KT[:, m, th * 512:(th + 1) * 512], ps)
                for kt4 in range(4):  # V for the 4 k-tiles in this quarter
                    ktg = th * 4 + kt4
                    for vb in range(2):
                        ps = psA2.tile([P, 512], f32, name="psA2_t")
                        for ci in range(CI):
                            nc.tensor.matmul(
                                ps,
                                lhsT=xt_sb[:, ci, kt4 * P:(kt4 + 1) * P],
                                rhs=wkv_sb[:, ci, C + vb * 512:C + (vb + 1) * 512],
                                start=(ci == 0),
                                stop=(ci == CI - 1),
                            )
                        nc.vector.tensor_copy(
                            V[:, ktg, vb * 8:(vb + 1) * 8, :],
                            ps.rearrange("p (h d) -> p h d", d=HD),
                        )

        # ---------------- Phase B: attention ----------------
        with tc.tile_pool(name="psS", bufs=3, space="PSUM") as psS, \
             tc.tile_pool(name="psO", bufs=2, space="PSUM") as psO, \
             tc.tile_pool(name="psL", bufs=2, space="PSUM") as psL, \
             tc.tile_pool(name="pP", bufs=6) as pP, \
             tc.tile_pool(name="pR", bufs=4) as pR, \
             tc.tile_pool(name="pRB", bufs=2) as pRB, \
             tc.tile_pool(name="pRD", bufs=4, space="DRAM") as pRD:
            for m in range(CI):  # head pair (2m, 2m+1)
                for qb in range(NQB):
                    nkt = 4 * (qb + 1)  # causal k-extent for this q-block
                    o_ps = psO.tile([P, QB], f32, name="o_ps")
                    l_ps = psL.tile([P, QB], f32, name="l_ps")
                    for kt in range(nkt):
                        for hh in range(2):
                            h = 2 * m + hh
                            hp = hh * 64
                            s_ps = psS.tile([P, QB], f32, name="s_ps")
                            nc.tensor.matmul(
                                s_ps,
                                lhsT=KT[hp:hp + 64, m, kt * P:(kt + 1) * P],
                                rhs=QT[hp:hp + 64, m, qb * QB:(qb + 1) * QB],
                                start=True,
                                stop=True,
                            )
                            pt = pP.tile([P, QB], bf16, name="pt")
                            nc.scalar.activation(pt, s_ps, EXP, scale=0.125)
                            if kt >= nkt - 4:
                                nc.vector.tensor_mul(pt, pt, maskS[:, kt - (nkt - 4)])
                            nc.tensor.matmul(
                                o_ps[hp:hp + 64],
                                lhsT=V[:, kt, h, :],
                                rhs=pt,
                                start=(kt == 0),
                                stop=(kt == nkt - 1),
                                tile_position=(0, hp),
                                skip_group_check=True,
                            )
                            nc.tensor.matmul(
                                l_ps[64 - hp:65 - hp],
                                lhsT=ones_col,
                                rhs=pt,
                                start=(kt == 0),
                                stop=(kt == nkt - 1),
                                tile_position=(0, 64 - hp),
                                skip_group_check=True,
                            )
                    for hh in range(2):  # normalize: y^T = O^T * (1/l) per head
                        hp = hh * 64
                        r_sb = pR.tile([1, QB], f32, name="r_sb")
                        nc.vector.reciprocal(r_sb, l_ps[64 - hp:65 - hp])
                        r_dr = pRD.tile([1, QB], f32, name="r_dr")
                        nc.sync.dma_start(r_dr, r_sb)
                        rb_sb = pRB.tile([P, QB], f32, name="rb_sb")
                        nc.sync.dma_start(
                            rb_sb[hp:hp + 64], r_dr.to_broadcast((64, QB))
                        )
                        nc.vector.tensor_mul(
                            yT[hp:hp + 64, m, qb * QB:(qb + 1) * QB],
                            o_ps[hp:hp + 64],
                            rb_sb[hp:hp + 64],
                        )

        # ---------------- Phase C: output projection (float32r) ----------------
        with tc.tile_pool(name="wpp", bufs=1) as wpp, \
             tc.tile_pool(name="osb", bufs=3) as osb, \
             tc.tile_pool(name="psC", bufs=4, space="PSUM") as psC:
            wp_sb = wpp.tile([P, CI, C], f32r, name="wp_sb")
            nc.sync.dma_start(wp_sb, wp.rearrange("(o p) f -> p o f", p=P))
            out_r = out_t.rearrange("(o p) q -> p o q", p=P)
            for co in range(CI):
                for q2 in range(TQ // 512):
                    ps = psC.tile([P, 512], f32, name="psC_t")
                    for ci in range(CI):
                        nc.tensor.matmul(
                            ps,
                            lhsT=wp_sb[:, ci, co * P:(co + 1) * P],
                            rhs=yT[:, ci, q2 * 512:(q2 + 1) * 512],
                            start=(ci == 0),
                            stop=(ci == CI - 1),
                        )
                    o_sb = osb.tile([P, 512], f32, name="o_sb")
                    nc.vector.tensor_copy(o_sb, ps)
                    nc.sync.dma_start(out_r[:, co, q2 * 512:(q2 + 1) * 512], o_sb)

    nc.compile()
    return nc


def _prep_inputs(x, W_attn, W_proj):
    """Host-side shard/layout prep. Pure data movement + dtype casts."""
    x = np.asarray(x, dtype=np.float32)
    wa_bf = np.ascontiguousarray(W_attn, dtype=np.float32).astype(BF16)
    wp_f = np.ascontiguousarray(W_proj, dtype=np.float32)
    # Causal mask in S^T layout [k within 512-span, q within 256-block]:
    # local query i <-> global 2*(qb*256+i)+p; key kk global qb*512+kk.
    # Valid iff kk <= 2*i + p.  Same tile for every q-block.
    kk = np.arange(4 * P)[:, None]
    ii = np.arange(QB)[None, :]
    masks = [(kk <= 2 * ii + p).astype(np.float32).astype(BF16) for p in (0, 1)]

    in_maps = []
    for c in range(NCORE):
        b, p = c // 2, c % 2
        xt_bf = np.ascontiguousarray(x[b].T).astype(BF16)
        xtq_bf = np.ascontiguousarray(x[b, p::2, :].T).astype(BF16)
        in_maps.append(
            {
                "xt": xt_bf,
                "xtq": xtq_bf,
                "wa": wa_bf,
                "wp": wp_f,
                "mk": masks[p],
            }
        )
    return in_maps


def kernel(x, W_attn, W_proj):
    global LAST_RESULTS
    from concourse.bass_utils import run_bass_kernel_spmd

    if "nc" not in _CACHE:
        _CACHE["nc"] = _build()
    nc = _CACHE["nc"]

    in_maps = _prep_inputs(x, W_attn, W_proj)
    trace = os.environ.get("CC_TRACE", "0") == "1"
    res = run_bass_kernel_spmd(
        nc, in_maps, core_ids=list(range(NCORE)), trace=trace
    )
    LAST_RESULTS = res

    out = np.empty((B, T, C), dtype=np.float32)
    for c in range(NCORE):
        b, p = c // 2, c % 2
        out[b, p::2, :] = res.results[c]["out_t"].T
    return out
